# revision 33
# baseline (speedup 1.0000x reference)
"""GCN layer (CrossViewGCN layer 1) on 8 Trainium2 NeuronCores.

Reference computation (shapes hardcoded):
    X = input[:, :512]                      # [8192, 512]
    A = input[:, 512:8704] + I              # [8192, 8192]
    d = colsum(A); Dh = diag(d^-1/2)
    support = X @ W                         # [8192, 256]
    out_mm  = Dh @ A @ Dh @ support         # [8192, 256]
    return concat(out_mm, A)                # [8192, 8448]

Sharding: 1D row partition of A / output across the 8 cores (1024 rows
each). The diagonal scalings are folded into the small operands and the
bulk matmul is decomposed around its means so the device does a single
fp8 DoubleRow GEMM per core while all mean terms stay exact f32:

    S    = d^-1/2[:, None] * (X @ W)                  (host, [8192, 256])
    A+I  = a0*J + dA,  S = mu + dS   (a0 = 0.5, mu = colmean(S))
    out_mm rows_i = d^-1/2[rows_i] * ( a0*colsum(S)
                                     + rowsum(dA_i)*mu
                                     + dA_i @ dS )    (device: dA_i @ dS)

dA in [-0.5, 0.5] and dS (zero-mean) are an order of magnitude smaller
than A and S, so quantizing them to fp8e4m3 yields ~7e-6 global
relative error (better than a direct bf16 GEMM) while DoubleRow packs a
256-deep contraction per matmul.

Device-side layouts are partition-major ([128, slab, free]) so every
DMA is 128 long contiguous runs — fast HWDGE descriptor generation.
DMA issue alternates between the two HWDGE rings (SP via nc.sync, ACT
via nc.scalar) so transfers on the two rings overlap.
"""

import numpy as np
import ml_dtypes

NSMP = 8192
NA = 512
DOUT = 256
REALNA = 520
NCORES = 8
ROWS = NSMP // NCORES  # 1024 output rows per core
P = 128
KSLABS = NSMP // P  # 64 contraction slabs of 128
KPAIRS = KSLABS // 2  # 32 DoubleRow slab-pairs (256-deep each)
S_GRP = 8  # k-slabs per S chunk DMA
A_GRP = 8  # k-slabs per streamed dA^T DMA group (1 MiB fp8)
MM_N = 512  # output free dim per matmul (= one PSUM bank of f32)

A0 = np.float32(0.5)  # mean removed from A+I before fp8 quantization

_compiled = None
last_results = None  # BassKernelResults of the most recent run (for harnesses)


def _get_compiled():
    global _compiled
    if _compiled is not None:
        return _compiled

    import concourse.bacc as bacc
    import concourse.mybir as mybir
    import concourse.tile as tile

    fp8 = mybir.dt.float8e4
    f32 = mybir.dt.float32
    DR = mybir.MatmulPerfMode.DoubleRow

    nc = bacc.Bacc(
        "TRN2", target_bir_lowering=False, debug=False, num_devices=NCORES
    )
    # partition-major: at[p, t, m] = dA_i^T[t*128 + p, m]
    at = nc.dram_tensor("at", [P, KSLABS, ROWS], fp8, kind="ExternalInput")
    # partition-major: s[p, t, n] = dS[t*128 + p, n]
    s = nc.dram_tensor("s", [P, KSLABS, DOUT], fp8, kind="ExternalInput")
    ot = nc.dram_tensor("ot", [DOUT, ROWS], f32, kind="ExternalOutput")

    n_schunk = KSLABS // S_GRP  # 8
    # dA groups: small first groups so the PE starts ~2 us after the DMA
    # rings open, then 4-slab (0.5 MiB) groups for fine-grained just-in-time
    # delivery. The A stream alone demands ~260 GB/s against a ~340 GB/s
    # aggregate wire, so delivery order must match consumption order.
    a_groups = [2, 2] + [4] * 15  # slab counts, sum = 64, all even

    with tile.TileContext(nc) as tc:
        with (
            tc.tile_pool(name="s_pool", bufs=1) as s_pool,
            tc.tile_pool(name="a_pool", bufs=10) as a_pool,
            tc.tile_pool(name="o_pool", bufs=1) as o_pool,
            tc.tile_pool(name="ps_pool", bufs=1, space="PSUM") as ps_pool,
        ):
            # dS stays resident: 8 chunks x [128, 8, 256] fp8 (2 MiB total),
            # loaded in consumption order alongside the dA stream.
            s_tiles = [None] * n_schunk

            def load_s_chunk(c, eng):
                s_t = s_pool.tile(
                    [P, S_GRP, DOUT], fp8, name=f"s_t{c}", tag=f"s_t{c}"
                )
                eng.dma_start(out=s_t[:], in_=s[:, c * S_GRP : (c + 1) * S_GRP, :])
                s_tiles[c] = s_t

            # OT = dS^T @ dA^T as two [128, 1024] f32 PSUM tiles
            # (n-tile j covers output rows j*128..(j+1)*128 of ot).
            ps = []
            for j in range(DOUT // P):
                ps_t = ps_pool.tile([P, ROWS], f32, name=f"ps{j}", tag=f"ps{j}")
                ps.append(ps_t)

            # Pre-warm the PE HAM clock gate during the preamble/DMA dead
            # time: ~4.5 us of dependency-free matmuls into a scratch PSUM
            # bank keep the PE busy through one 4096-cycle activity window,
            # so the real stream starts at 2.4 GHz instead of 1.2 GHz.
            warm_in = o_pool.tile([P, 2, MM_N], fp8, name="warm_in", tag="warm_in")
            warm_ps = ps_pool.tile([P, MM_N], f32, name="warm_ps", tag="warm_ps")
            warm_act = o_pool.tile([P, 8], f32, name="warm_act", tag="warm_act")
            nc.gpsimd.memset(warm_in[:1, :1, :1], 0.0)
            # hoist the ACT activation-table load into the preamble dead time
            # so the tail's scalar-engine PSUM eviction doesn't pay it
            nc.scalar.copy(warm_act[:], warm_in[:, 0, :8])
            for _ in range(15):
                nc.tensor.matmul(
                    warm_ps[:],
                    warm_in[:, :, :P],
                    warm_in[:],
                    start=True,
                    stop=True,
                    perf_mode=DR,
                )

            # Merge S-chunk and dA-group DMAs into one deadline-ordered list
            # (s_c is consumed starting at slab S_GRP*c), then alternate rings
            # item-by-item: each ring's FIFO is then also deadline-ordered
            # and the two rings share the wire ~evenly.
            items = []
            si = 0
            off = 0
            for u, grp in enumerate(a_groups):
                while si < n_schunk and S_GRP * si < off + grp:
                    items.append(("s", si, 0))
                    si += 1
                items.append(("a", u, off))
                off += grp

            for idx, (kind, u, off) in enumerate(items):
                # the two launch-critical items (s0, first dA group) go on the
                # SP ring — the ACT ring starts ~1.3 us late behind the
                # hoisted activation-table load; alternate afterwards
                if idx < 2:
                    eng = nc.sync
                else:
                    eng = nc.scalar if idx % 2 == 0 else nc.sync
                if kind == "s":
                    load_s_chunk(u, eng)
                    continue
                grp = a_groups[u]
                a_t = a_pool.tile([P, grp, ROWS], fp8, name="a_t", tag="a_t")
                eng.dma_start(out=a_t[:], in_=at[:, off : off + grp, :])
                for g in range(0, grp, 2):
                    t = off + g  # first slab of the pair
                    q = t // 2  # DoubleRow pair index
                    sc = s_tiles[t // S_GRP]
                    sl = t % S_GRP
                    for j in range(DOUT // P):
                        # [128, 2, 128] stationary: two k-slabs per PE cell
                        lhsT = sc[:, sl : sl + 2, j * P : (j + 1) * P]
                        for mc in range(ROWS // MM_N):
                            nc.tensor.matmul(
                                ps[j][:, mc * MM_N : (mc + 1) * MM_N],
                                lhsT,
                                a_t[:, g : g + 2, mc * MM_N : (mc + 1) * MM_N],
                                start=(q == 0),
                                stop=(q == KPAIRS - 1),
                                perf_mode=DR,
                            )

            # evict per PSUM bank so each copy/DMA overlaps the final matmuls
            # of the other banks
            for j in range(DOUT // P):
                for mc in range(ROWS // MM_N):
                    o_t = o_pool.tile(
                        [P, MM_N], f32, name=f"o_t{j}_{mc}", tag=f"o_t{j}_{mc}"
                    )
                    src = ps[j][:, mc * MM_N : (mc + 1) * MM_N]
                    # the last bank to finish goes to the ACT engine so the
                    # DVE (3 copies) and ACT (1 copy) drain in parallel
                    if (j, mc) == (1, 1):
                        nc.scalar.copy(o_t[:], src)
                    else:
                        nc.vector.tensor_copy(o_t[:], src)
                    # spread the final write-backs over both HWDGE rings so
                    # their completion receipts overlap
                    oeng = nc.sync if j == 0 else nc.scalar
                    oeng.dma_start(
                        out=ot[j * P : (j + 1) * P, mc * MM_N : (mc + 1) * MM_N],
                        in_=o_t[:],
                    )

    nc.compile()
    _compiled = nc
    return _compiled


def kernel(input, weight):
    global last_results
    input = np.asarray(input, dtype=np.float32)
    weight = np.asarray(weight, dtype=np.float32)

    X = input[:, :NA]
    A = input[:, REALNA - 8 : REALNA - 8 + NSMP]  # [8192, 8192] view (no +I yet)

    # d = colsum(A + I); the identity adds exactly 1 to every column sum.
    d = A.sum(axis=0, dtype=np.float64) + 1.0
    dinv = (1.0 / np.sqrt(d)).astype(np.float32)  # [8192]
    # rowsum(dA) = rowsum(A + I) - a0*8192, needed for the mean correction
    rowsum_dA = (A.sum(axis=1, dtype=np.float64) + 1.0 - float(A0) * NSMP).astype(
        np.float32
    )

    support = X @ weight  # [8192, 256] f32
    S = support * dinv[:, None]
    mu = S.mean(axis=0, dtype=np.float64).astype(np.float32)  # [256]
    colsum_S = S.sum(axis=0, dtype=np.float64).astype(np.float32)  # [256]
    dS = (S - mu[None, :]).astype(ml_dtypes.float8_e4m3)
    # partition-major [128, 64, 256]
    s_dev = np.ascontiguousarray(dS.reshape(KSLABS, P, DOUT).swapaxes(0, 1))

    diag = np.arange(ROWS)
    in_maps = []
    for i in range(NCORES):
        blk = A[i * ROWS : (i + 1) * ROWS, :]  # [1024, 8192] view
        at_i = (blk.T - A0).astype(ml_dtypes.float8_e4m3)  # [8192, 1024]
        grows = i * ROWS + diag
        # fold the +I into this block's transposed, centered copy
        at_i[grows, diag] = (blk[diag, grows] + (1.0 - A0)).astype(
            ml_dtypes.float8_e4m3
        )
        # partition-major [128, 64, 1024]
        at_dev = np.ascontiguousarray(at_i.reshape(KSLABS, P, ROWS).swapaxes(0, 1))
        in_maps.append({"at": at_dev, "s": s_dev})

    # If BASS_TRACE is set but the axon NTFF hook module is absent, the
    # bass_utils trace path would die on import; provide a no-op hook so it
    # degrades to an untraced run instead.
    try:
        import antenv.axon_hooks  # noqa: F401
    except Exception:
        import sys
        import types

        _m = types.ModuleType("antenv.axon_hooks")
        _m.get_axon_ntff_profile_hook = lambda: None
        _m.set_axon_ntff_profile_hook = lambda h: None
        sys.modules["antenv.axon_hooks"] = _m

    from concourse.bass_utils import run_bass_kernel_spmd

    nc = _get_compiled()
    res = run_bass_kernel_spmd(nc, in_maps, list(range(NCORES)))
    last_results = res

    out = np.empty((NSMP, DOUT + NSMP), dtype=np.float32)
    out[:, DOUT:] = A
    gr = np.arange(NSMP)
    out[gr, DOUT + gr] += 1.0
    # exact mean terms: a0*colsum(S) + rowsum(dA)[:, None] * mu
    mean_terms = float(A0) * colsum_S[None, :] + rowsum_dA[:, None] * mu[None, :]
    for i in range(NCORES):
        ot_i = res.results[i]["ot"]  # [256, 1024] f32 = (dA_i @ dS)^T
        rows = slice(i * ROWS, (i + 1) * ROWS)
        out[rows, :DOUT] = (ot_i.T + mean_terms[rows]) * dinv[rows, None]
    return out


# revision 36
# speedup vs baseline: 1.1085x; 1.1085x over previous
"""GCN layer (CrossViewGCN layer 1) on 8 Trainium2 NeuronCores.

Reference computation (shapes hardcoded):
    X = input[:, :512]                      # [8192, 512]
    A = input[:, 512:8704] + I              # [8192, 8192]
    d = colsum(A); Dh = diag(d^-1/2)
    support = X @ W                         # [8192, 256]
    out_mm  = Dh @ A @ Dh @ support         # [8192, 256]
    return concat(out_mm, A)                # [8192, 8448]

Sharding: 1D row partition of A / output across the 8 cores (1024 rows
each). The diagonal scalings are folded into the small operands and the
bulk matmul is decomposed around its means so the device does a single
fp8 DoubleRow GEMM per core while all mean terms stay exact f32:

    S    = d^-1/2[:, None] * (X @ W)                  (host, [8192, 256])
    A+I  = a0*J + dA,  S = mu + dS   (a0 = 0.5, mu = colmean(S))
    out_mm rows_i = d^-1/2[rows_i] * ( a0*colsum(S)
                                     + rowsum(dA_i)*mu
                                     + dA_i @ dS )    (device: dA_i @ dS)

dA in [-0.5, 0.5] and dS (zero-mean) are an order of magnitude smaller
than A and S, so quantizing them to fp8e4m3 yields ~7e-6 global
relative error (better than a direct bf16 GEMM) while DoubleRow packs a
256-deep contraction per matmul.

Device-side layouts are partition-major ([128, slab, free]) so every
DMA is 128 long contiguous runs — fast HWDGE descriptor generation.
DMA issue alternates between the two HWDGE rings (SP via nc.sync, ACT
via nc.scalar) so transfers on the two rings overlap.
"""

import numpy as np
import ml_dtypes

NSMP = 8192
NA = 512
DOUT = 256
REALNA = 520
NCORES = 8
ROWS = NSMP // NCORES  # 1024 output rows per core
P = 128
KSLABS = NSMP // P  # 64 contraction slabs of 128
KPAIRS = KSLABS // 2  # 32 DoubleRow slab-pairs (256-deep each)
S_GRP = 8  # k-slabs per S chunk DMA
A_GRP = 8  # k-slabs per streamed dA^T DMA group (1 MiB fp8)
MM_N = 512  # output free dim per matmul (= one PSUM bank of f32)

A0 = np.float32(0.5)  # mean removed from A+I before fp8 quantization

_compiled = None
last_results = None  # BassKernelResults of the most recent run (for harnesses)


def _get_compiled():
    global _compiled
    if _compiled is not None:
        return _compiled

    import concourse.bacc as bacc
    import concourse.mybir as mybir
    import concourse.tile as tile

    fp8 = mybir.dt.float8e4
    f32 = mybir.dt.float32
    DR = mybir.MatmulPerfMode.DoubleRow

    nc = bacc.Bacc(
        "TRN2", target_bir_lowering=False, debug=False, num_devices=NCORES
    )
    # partition-major: at[p, t, m] = dA_i^T[t*128 + p, m]
    at = nc.dram_tensor("at", [P, KSLABS, ROWS], fp8, kind="ExternalInput")
    # partition-major: s[p, t, n] = dS[t*128 + p, n]
    s = nc.dram_tensor("s", [P, KSLABS, DOUT], fp8, kind="ExternalInput")
    ot = nc.dram_tensor("ot", [DOUT, ROWS], f32, kind="ExternalOutput")

    n_schunk = KSLABS // S_GRP  # 8
    # dA groups: small first groups so the PE starts ~2 us after the DMA
    # rings open, then 4-slab (0.5 MiB) groups for fine-grained just-in-time
    # delivery. The A stream alone demands ~260 GB/s against a ~340 GB/s
    # aggregate wire, so delivery order must match consumption order.
    a_groups = [2, 2] + [4] * 15  # slab counts, sum = 64, all even

    with tile.TileContext(nc) as tc:
        with (
            tc.tile_pool(name="s_pool", bufs=1) as s_pool,
            tc.tile_pool(name="a_pool", bufs=10) as a_pool,
            tc.tile_pool(name="o_pool", bufs=1) as o_pool,
            tc.tile_pool(name="ps_pool", bufs=1, space="PSUM") as ps_pool,
        ):
            # dS stays resident: 8 chunks x [128, 8, 256] fp8 (2 MiB total),
            # loaded in consumption order alongside the dA stream.
            s_tiles = [None] * n_schunk

            def load_s_chunk(c, eng):
                s_t = s_pool.tile(
                    [P, S_GRP, DOUT], fp8, name=f"s_t{c}", tag=f"s_t{c}"
                )
                eng.dma_start(out=s_t[:], in_=s[:, c * S_GRP : (c + 1) * S_GRP, :])
                s_tiles[c] = s_t

            # OT = dS^T @ dA^T as two [128, 1024] f32 PSUM tiles
            # (n-tile j covers output rows j*128..(j+1)*128 of ot).
            ps = []
            for j in range(DOUT // P):
                ps_t = ps_pool.tile([P, ROWS], f32, name=f"ps{j}", tag=f"ps{j}")
                ps.append(ps_t)

            # Pre-warm the PE HAM clock gate during the preamble/DMA dead
            # time: ~4.5 us of dependency-free matmuls into a scratch PSUM
            # bank keep the PE busy through one 4096-cycle activity window,
            # so the real stream starts at 2.4 GHz instead of 1.2 GHz.
            warm_in = o_pool.tile([P, 2, MM_N], fp8, name="warm_in", tag="warm_in")
            warm_ps = ps_pool.tile([P, MM_N], f32, name="warm_ps", tag="warm_ps")
            nc.gpsimd.memset(warm_in[:1, :1, :1], 0.0)
            for _ in range(15):
                nc.tensor.matmul(
                    warm_ps[:],
                    warm_in[:, :, :P],
                    warm_in[:],
                    start=True,
                    stop=True,
                    perf_mode=DR,
                )

            # Merge S-chunk and dA-group DMAs into one deadline-ordered list
            # (s_c is consumed starting at slab S_GRP*c), then alternate rings
            # item-by-item: each ring's FIFO is then also deadline-ordered
            # and the two rings share the wire ~evenly.
            items = []
            si = 0
            off = 0
            for u, grp in enumerate(a_groups):
                while si < n_schunk and S_GRP * si < off + grp:
                    items.append(("s", si, 0))
                    si += 1
                items.append(("a", u, off))
                off += grp

            for idx, (kind, u, off) in enumerate(items):
                eng = nc.sync if idx % 2 == 0 else nc.scalar
                if kind == "s":
                    load_s_chunk(u, eng)
                    continue
                grp = a_groups[u]
                a_t = a_pool.tile([P, grp, ROWS], fp8, name="a_t", tag="a_t")
                eng.dma_start(out=a_t[:], in_=at[:, off : off + grp, :])
                for g in range(0, grp, 2):
                    t = off + g  # first slab of the pair
                    q = t // 2  # DoubleRow pair index
                    sc = s_tiles[t // S_GRP]
                    sl = t % S_GRP
                    for j in range(DOUT // P):
                        # [128, 2, 128] stationary: two k-slabs per PE cell
                        lhsT = sc[:, sl : sl + 2, j * P : (j + 1) * P]
                        for mc in range(ROWS // MM_N):
                            nc.tensor.matmul(
                                ps[j][:, mc * MM_N : (mc + 1) * MM_N],
                                lhsT,
                                a_t[:, g : g + 2, mc * MM_N : (mc + 1) * MM_N],
                                start=(q == 0),
                                stop=(q == KPAIRS - 1),
                                perf_mode=DR,
                            )

            # evict per PSUM bank so each copy/DMA overlaps the final matmuls
            # of the other banks
            for j in range(DOUT // P):
                for mc in range(ROWS // MM_N):
                    o_t = o_pool.tile(
                        [P, MM_N], f32, name=f"o_t{j}_{mc}", tag=f"o_t{j}_{mc}"
                    )
                    nc.vector.tensor_copy(
                        o_t[:], ps[j][:, mc * MM_N : (mc + 1) * MM_N]
                    )
                    # spread the final write-backs over both HWDGE rings so
                    # their completion receipts overlap
                    oeng = nc.sync if j == 0 else nc.scalar
                    oeng.dma_start(
                        out=ot[j * P : (j + 1) * P, mc * MM_N : (mc + 1) * MM_N],
                        in_=o_t[:],
                    )

    nc.compile()
    _compiled = nc
    return _compiled


def kernel(input, weight):
    global last_results
    input = np.asarray(input, dtype=np.float32)
    weight = np.asarray(weight, dtype=np.float32)

    X = input[:, :NA]
    A = input[:, REALNA - 8 : REALNA - 8 + NSMP]  # [8192, 8192] view (no +I yet)

    # d = colsum(A + I); the identity adds exactly 1 to every column sum.
    d = A.sum(axis=0, dtype=np.float64) + 1.0
    dinv = (1.0 / np.sqrt(d)).astype(np.float32)  # [8192]
    # rowsum(dA) = rowsum(A + I) - a0*8192, needed for the mean correction
    rowsum_dA = (A.sum(axis=1, dtype=np.float64) + 1.0 - float(A0) * NSMP).astype(
        np.float32
    )

    support = X @ weight  # [8192, 256] f32
    S = support * dinv[:, None]
    mu = S.mean(axis=0, dtype=np.float64).astype(np.float32)  # [256]
    colsum_S = S.sum(axis=0, dtype=np.float64).astype(np.float32)  # [256]
    dS = (S - mu[None, :]).astype(ml_dtypes.float8_e4m3)
    # partition-major [128, 64, 256]
    s_dev = np.ascontiguousarray(dS.reshape(KSLABS, P, DOUT).swapaxes(0, 1))

    diag = np.arange(ROWS)
    in_maps = []
    for i in range(NCORES):
        blk = A[i * ROWS : (i + 1) * ROWS, :]  # [1024, 8192] view
        at_i = (blk.T - A0).astype(ml_dtypes.float8_e4m3)  # [8192, 1024]
        grows = i * ROWS + diag
        # fold the +I into this block's transposed, centered copy
        at_i[grows, diag] = (blk[diag, grows] + (1.0 - A0)).astype(
            ml_dtypes.float8_e4m3
        )
        # partition-major [128, 64, 1024]
        at_dev = np.ascontiguousarray(at_i.reshape(KSLABS, P, ROWS).swapaxes(0, 1))
        in_maps.append({"at": at_dev, "s": s_dev})

    # If BASS_TRACE is set but the axon NTFF hook module is absent, the
    # bass_utils trace path would die on import; provide a no-op hook so it
    # degrades to an untraced run instead.
    try:
        import antenv.axon_hooks  # noqa: F401
    except Exception:
        import sys
        import types

        _m = types.ModuleType("antenv.axon_hooks")
        _m.get_axon_ntff_profile_hook = lambda: None
        _m.set_axon_ntff_profile_hook = lambda h: None
        sys.modules["antenv.axon_hooks"] = _m

    from concourse.bass_utils import run_bass_kernel_spmd

    nc = _get_compiled()
    res = run_bass_kernel_spmd(nc, in_maps, list(range(NCORES)))
    last_results = res

    out = np.empty((NSMP, DOUT + NSMP), dtype=np.float32)
    out[:, DOUT:] = A
    gr = np.arange(NSMP)
    out[gr, DOUT + gr] += 1.0
    # exact mean terms: a0*colsum(S) + rowsum(dA)[:, None] * mu
    mean_terms = float(A0) * colsum_S[None, :] + rowsum_dA[:, None] * mu[None, :]
    for i in range(NCORES):
        ot_i = res.results[i]["ot"]  # [256, 1024] f32 = (dA_i @ dS)^T
        rows = slice(i * ROWS, (i + 1) * ROWS)
        out[rows, :DOUT] = (ot_i.T + mean_terms[rows]) * dinv[rows, None]
    return out
